# revision 14
# baseline (speedup 1.0000x reference)
"""Attention-pooling (ContextLayer) Trainium2 Bass kernel, 8-core SPMD.

Computes, for full inputs:
    scores  = einsum('qd,bsd->bqs', query, context) + (1-mask)*NEG
    weights = softmax(scores, axis=-1)
    out     = einsum('bqs,bsd->bqd', weights, context)

Sharding: data-parallel over batch (16 batches -> 8 cores x 2), query
replicated. Per-core kernel is a single-pass (context read from HBM once)
streaming implementation:
  - context streamed in 8 chunks of 512 rows per batch, natural layout,
    retained in SBUF for the second matmul
  - PE transposes each chunk (D-on-partitions copy) for the scores matmul
  - scores matmul (fp32r) with query^T stationary; mask folded in as a
    K=1 bf16 accumulation row (ones^T @ (1-m)*NEG)
  - per-chunk scores copy to SBUF (ACT) + partial max (DVE)
  - exp(x-max) with fused sum on ACT, reciprocal on DVE
  - second matmul (fp32r) accumulates over all chunks in PSUM, normalized
    by 1/sum at the end.

fp32r notes: walrus requires every fp32r-matmul input to be *produced* as
float32r, so all matmul-feeding tiles are float32r and the PSUM->SBUF
copies perform the rounding; DMA-landed tiles (cn, qn) are declared
float32r and loaded via SWDGE with a nominal fp32->fp32r cast.
"""

import numpy as np

import concourse.bass as bass
import concourse.mybir as mybir
import concourse.tile as tile
from concourse import bacc
from concourse.bass_utils import run_bass_kernel_spmd
from concourse.masks import make_identity

# Problem shapes (hardcoded per contract)
B, S, D, Q = 16, 4096, 768, 64
NCORES = 8
BPC = B // NCORES          # batches per core
NEG = -100000.0

CH = 512                   # S-chunk rows per DMA
NCH = S // CH              # 8 chunks per batch
NSUB = CH // 128           # 4 128-row subtiles per chunk
KD = D // 128              # 6 contraction chunks over D
HALF = 256                 # half-chunk columns for transposed-context tiles
CN_BUFS = 10               # retained natural-context chunk slots (8 + prefetch)

F32 = mybir.dt.float32
BF16 = mybir.dt.bfloat16
MM_DT = mybir.dt.float32r  # matmul compute dtype (fp32 bits, fast PE path)


def build_nc():
    nc = bacc.Bacc("TRN2", target_bir_lowering=False, debug=False)

    ctx_d = nc.dram_tensor("context", [BPC, S, D], F32, kind="ExternalInput")
    msk_d = nc.dram_tensor("attention_mask", [BPC, S], F32, kind="ExternalInput")
    qry_d = nc.dram_tensor("query", [Q, D], F32, kind="ExternalInput")
    out_d = nc.dram_tensor("out", [BPC, Q, D], F32, kind="ExternalOutput")

    AX = mybir.AxisListType
    OP = mybir.AluOpType
    ACTF = mybir.ActivationFunctionType

    with tile.TileContext(nc) as tc:
        with (
            tc.tile_pool(name="const", bufs=1) as constp,
            tc.tile_pool(name="cn", bufs=CN_BUFS) as cnp,
            tc.tile_pool(name="ct", bufs=2) as ctp,
            tc.tile_pool(name="scores", bufs=2) as scp,
            tc.tile_pool(name="wt", bufs=2) as wtp,
            tc.tile_pool(name="small", bufs=2) as smp,
            tc.tile_pool(name="ps_t", bufs=2, space="PSUM") as ps_t,
            tc.tile_pool(name="ps_s", bufs=2, space="PSUM") as ps_s,
            tc.tile_pool(name="ps_o", bufs=1, space="PSUM") as ps_o,
            tc.tile_pool(name="ps_w", bufs=2, space="PSUM") as ps_w,
        ):
            ident_f = constp.tile([128, 128], F32, tag="ident_f")
            make_identity(nc, ident_f[:])
            ident = constp.tile([128, 128], MM_DT, tag="ident")
            nc.vector.tensor_copy(ident[:], ident_f[:])

            # query^T: [128, 64] x6 built by PE transpose (fp32r)
            qn = constp.tile([Q, D], MM_DT, tag="qn")
            nc.sync.dma_start(qn[:], qry_d.ap().bitcast(MM_DT))
            qt = constp.tile([128, KD, Q], MM_DT, tag="qt")
            for k in range(KD):
                pq = ps_w.tile([128, Q], MM_DT, tag="wp")
                nc.tensor.transpose(
                    pq[:], qn[:, k * 128 : (k + 1) * 128], ident[:Q, :Q]
                )
                nc.vector.tensor_copy(qt[:, k, :], pq[:])

            # additive mask on one partition, folded into matmul1 as a K=1
            # bf16 accumulation row: scores += ones[1,Q]^T @ madd[1,S]
            madd = constp.tile([1, BPC * S], BF16, tag="madd")
            nc.gpsimd.dma_start(
                madd[:], msk_d.ap().rearrange("b s -> (b s)").unsqueeze(0)
            )  # f32 -> bf16 cast, simple 2D AP
            # in-place: (1-m)*NEG == m*(-NEG) + NEG  (exactly 0 for m==1; -1e5
            # rounds to -99840 in bf16 — same all-underflow contribution)
            nc.vector.tensor_scalar(
                madd[:], madd[:], -NEG, NEG, op0=OP.mult, op1=OP.add
            )
            ones = constp.tile([1, Q], BF16, tag="ones")
            nc.vector.memset(ones[:], 1.0)

            for b in range(BPC):
                ssb = scp.tile([Q, S], F32, tag="sc_sb")
                pmax = smp.tile([Q, NCH], F32, tag="pmax")
                cn_tiles = []
                for c in range(NCH):
                    cn = cnp.tile([128, NSUB, D], MM_DT, tag="cn")
                    cn_tiles.append(cn)
                    src = ctx_d.ap()[b, c * CH : (c + 1) * CH, :].rearrange(
                        "(n p) d -> p n d", p=128
                    )
                    nc.sync.dma_start(cn[:], src.bitcast(MM_DT))

                    sc = ps_s.tile([Q, CH], F32, tag="sc")
                    for h in range(2):
                        ct = ctp.tile([128, KD, HALF], MM_DT, tag="ct")
                        for k in range(KD):
                            pt = ps_t.tile([128, HALF], MM_DT, tag="pt")
                            for j in range(2):
                                n = h * 2 + j
                                nc.tensor.transpose(
                                    pt[:, j * 128 : (j + 1) * 128],
                                    cn[:, n, k * 128 : (k + 1) * 128],
                                    ident[:],
                                )
                            if k % 2 == 0:
                                nc.vector.tensor_copy(ct[:, k, :], pt[:])
                            else:
                                nc.scalar.copy(ct[:, k, :], pt[:])
                        for k in range(KD):
                            nc.tensor.matmul(
                                sc[:, h * HALF : (h + 1) * HALF],
                                qt[:, k, :],
                                ct[:, k, :],
                                start=(k == 0),
                                stop=False,
                            )
                        # mask fold-in: K=1 row of ones^T @ madd (bf16)
                        base = b * S + c * CH + h * HALF
                        nc.tensor.matmul(
                            sc[:, h * HALF : (h + 1) * HALF],
                            ones[:],
                            madd[:, base : base + HALF],
                            start=False,
                            stop=True,
                        )
                    # copy scores chunk to SBUF (ACT) + partial max (DVE)
                    nc.scalar.copy(ssb[:, c * CH : (c + 1) * CH], sc[:])
                    nc.vector.reduce_max(pmax[:, c : c + 1], sc[:], axis=AX.X)

                # softmax: exp(s - max) with fused sum; weights unnormalized
                negm = smp.tile([Q, 1], F32, tag="negm")
                nc.vector.tensor_reduce(
                    out=negm[:], in_=pmax[:], axis=AX.X, op=OP.max, negate=True
                )
                esb = scp.tile([Q, S], MM_DT, tag="sc_sb")
                ssum = smp.tile([Q, 1], F32, tag="ssum")
                nc.scalar.activation(
                    esb[:], ssb[:], ACTF.Exp,
                    bias=negm[:], scale=1.0, accum_out=ssum[:],
                )
                rinv = smp.tile([Q, 1], F32, tag="rinv")
                nc.vector.reciprocal(rinv[:], ssum[:])

                # second matmul: out[q, d] = sum_s w[s, q] * ctx[s, d]
                op0 = ps_o.tile([Q, 384], F32, tag="op0")
                op1 = ps_o.tile([Q, 384], F32, tag="op1")
                for c in range(NCH):
                    cn = cn_tiles[c]
                    wp = ps_w.tile([128, NSUB * Q], MM_DT, tag="wp")
                    for n in range(NSUB):
                        nc.tensor.transpose(
                            wp[:, n * Q : (n + 1) * Q],
                            esb[:, (c * NSUB + n) * 128 : (c * NSUB + n + 1) * 128],
                            ident[:Q, :Q],
                        )
                    wt = wtp.tile([128, NSUB, Q], MM_DT, tag="wt")
                    if c % 2 == 0:
                        nc.vector.tensor_copy(
                            wt[:].rearrange("p n q -> p (n q)"), wp[:]
                        )
                    else:
                        nc.scalar.copy(
                            wt[:].rearrange("p n q -> p (n q)"), wp[:]
                        )
                    for n in range(NSUB):
                        first = (c == 0 and n == 0)
                        last = (c == NCH - 1 and n == NSUB - 1)
                        nc.tensor.matmul(
                            op0[:], wt[:, n, :], cn[:, n, 0:384],
                            start=first, stop=last,
                        )
                        nc.tensor.matmul(
                            op1[:], wt[:, n, :], cn[:, n, 384:768],
                            start=first, stop=last,
                        )

                osb = smp.tile([Q, D], F32, tag="osb")
                nc.vector.tensor_scalar(
                    osb[:, 0:384], op0[:], rinv[:], None, op0=OP.mult
                )
                nc.vector.tensor_scalar(
                    osb[:, 384:768], op1[:], rinv[:], None, op0=OP.mult
                )
                nc.sync.dma_start(out_d.ap()[b], osb[:])

    nc.compile()
    return nc


_NC_CACHE = None


def _get_nc():
    global _NC_CACHE
    if _NC_CACHE is None:
        _NC_CACHE = build_nc()
    return _NC_CACHE


def kernel(context, attention_mask, query):
    context = np.ascontiguousarray(np.asarray(context, dtype=np.float32))
    attention_mask = np.ascontiguousarray(np.asarray(attention_mask, dtype=np.float32))
    query = np.ascontiguousarray(np.asarray(query, dtype=np.float32))
    assert context.shape == (B, S, D)
    assert attention_mask.shape == (B, S)
    assert query.shape == (Q, D)

    nc = _get_nc()
    in_maps = [
        {
            "context": context[i * BPC : (i + 1) * BPC],
            "attention_mask": attention_mask[i * BPC : (i + 1) * BPC],
            "query": query,
        }
        for i in range(NCORES)
    ]
    res = run_bass_kernel_spmd(nc, in_maps, list(range(NCORES)))
    out = np.concatenate(
        [res.results[i]["out"] for i in range(NCORES)], axis=0
    ).astype(np.float32)
    return out


# revision 19
# speedup vs baseline: 1.4077x; 1.4077x over previous
"""Attention-pooling (ContextLayer) Trainium2 Bass kernel, 8-core SPMD.

Computes, for full inputs:
    scores  = einsum('qd,bsd->bqs', query, context) + (1-mask)*NEG
    weights = softmax(scores, axis=-1)
    out     = einsum('bqs,bsd->bqd', weights, context)

Sharding: data-parallel over batch (16 batches -> 8 cores x 2), query
replicated. Per-core kernel is a single-pass (context read from HBM once)
streaming implementation:
  - context streamed in 8 chunks of 512 rows per batch, natural layout,
    retained in SBUF for the second matmul
  - PE transposes each chunk (D-on-partitions copy) for the scores matmul
  - scores matmul (fp32r) with query^T stationary; mask folded in as a
    K=1 bf16 accumulation row (ones^T @ (1-m)*NEG)
  - per-chunk scores copy to SBUF (ACT) + partial max (DVE)
  - exp(x-max) with fused sum on ACT, reciprocal on DVE
  - second matmul (fp32r) accumulates over all chunks in PSUM, normalized
    by 1/sum at the end.

fp32r notes: walrus requires every fp32r-matmul input to be *produced* as
float32r, so all matmul-feeding tiles are float32r and the PSUM->SBUF
copies perform the rounding; DMA-landed tiles (cn, qn) are declared
float32r and loaded via SWDGE with a nominal fp32->fp32r cast.
"""

import numpy as np

import concourse.bass as bass
import concourse.mybir as mybir
import concourse.tile as tile
from concourse import bacc
from concourse.bass_utils import run_bass_kernel_spmd
from concourse.masks import make_identity

# Problem shapes (hardcoded per contract)
B, S, D, Q = 16, 4096, 768, 64
NCORES = 8
BPC = B // NCORES          # batches per core
NEG = -100000.0

CH = 512                   # S-chunk rows per DMA
NCH = S // CH              # 8 chunks per batch
NSUB = CH // 128           # 4 128-row subtiles per chunk
KD = D // 128              # 6 contraction chunks over D
CN_BUFS = 10               # retained natural-context chunk slots (8 + prefetch)

F32 = mybir.dt.float32
BF16 = mybir.dt.bfloat16
MM_DT = mybir.dt.float32r  # matmul compute dtype (fp32 bits, fast PE path)


def build_nc():
    nc = bacc.Bacc("TRN2", target_bir_lowering=False, debug=False)

    ctx_d = nc.dram_tensor("context", [BPC, S, D], F32, kind="ExternalInput")
    msk_d = nc.dram_tensor("attention_mask", [BPC, S], F32, kind="ExternalInput")
    qry_d = nc.dram_tensor("query", [Q, D], F32, kind="ExternalInput")
    out_d = nc.dram_tensor("out", [BPC, Q, D], F32, kind="ExternalOutput")

    AX = mybir.AxisListType
    OP = mybir.AluOpType
    ACTF = mybir.ActivationFunctionType

    with tile.TileContext(nc) as tc:
        with (
            tc.tile_pool(name="const", bufs=1) as constp,
            tc.tile_pool(name="cn", bufs=CN_BUFS) as cnp,
            tc.tile_pool(name="ct", bufs=2) as ctp,
            tc.tile_pool(name="scores", bufs=2) as scp,
            tc.tile_pool(name="wt", bufs=2) as wtp,
            tc.tile_pool(name="small", bufs=2) as smp,
            tc.tile_pool(name="ps_t", bufs=4, space="PSUM") as ps_t,
            tc.tile_pool(name="ps_s", bufs=2, space="PSUM") as ps_s,
            tc.tile_pool(name="ps_o", bufs=1, space="PSUM") as ps_o,
        ):
            ident_f = constp.tile([128, 128], F32, tag="ident_f")
            make_identity(nc, ident_f[:])
            ident = constp.tile([128, 128], MM_DT, tag="ident")
            nc.vector.tensor_copy(ident[:], ident_f[:])

            # query^T: [128, 64] x6 built by PE transpose (fp32r)
            qn = constp.tile([Q, D], MM_DT, tag="qn")
            nc.sync.dma_start(qn[:], qry_d.ap().bitcast(MM_DT))
            qt = constp.tile([128, KD, Q], MM_DT, tag="qt")
            for k in range(KD):
                pq = ps_t.tile([128, Q], MM_DT, tag="pt")
                nc.tensor.transpose(
                    pq[:], qn[:, k * 128 : (k + 1) * 128], ident[:Q, :Q]
                )
                nc.vector.tensor_copy(qt[:, k, :], pq[:])

            # additive mask on one partition, folded into matmul1 as a K=1
            # bf16 accumulation row: scores += ones[1,Q]^T @ madd[1,S]
            madd = constp.tile([1, BPC * S], BF16, tag="madd")
            nc.gpsimd.dma_start(
                madd[:], msk_d.ap().rearrange("b s -> (b s)").unsqueeze(0)
            )  # f32 -> bf16 cast, simple 2D AP
            # in-place: (1-m)*NEG == m*(-NEG) + NEG  (exactly 0 for m==1; -1e5
            # rounds to -99840 in bf16 — same all-underflow contribution)
            nc.vector.tensor_scalar(
                madd[:], madd[:], -NEG, NEG, op0=OP.mult, op1=OP.add
            )
            ones = constp.tile([1, Q], BF16, tag="ones")
            nc.vector.memset(ones[:], 1.0)

            for b in range(BPC):
                ssb = scp.tile([Q, S], F32, tag="sc_sb")
                pmax = smp.tile([Q, NCH], F32, tag="pmax")
                cn_tiles = []
                for c in range(NCH):
                    cn = cnp.tile([128, NSUB, D], MM_DT, tag="cn")
                    cn_tiles.append(cn)
                    src = ctx_d.ap()[b, c * CH : (c + 1) * CH, :].rearrange(
                        "(n p) d -> p n d", p=128
                    )
                    nc.sync.dma_start(cn[:], src.bitcast(MM_DT))

                    sc = ps_s.tile([Q, CH], F32, tag="sc")
                    ct = ctp.tile([128, KD, CH], MM_DT, tag="ct")
                    for k in range(KD):
                        pt = ps_t.tile([128, CH], MM_DT, tag="pt")
                        for n in range(NSUB):
                            nc.tensor.transpose(
                                pt[:, n * 128 : (n + 1) * 128],
                                cn[:, n, k * 128 : (k + 1) * 128],
                                ident[:],
                            )
                        if k % 2 == 0:
                            nc.vector.tensor_copy(ct[:, k, :], pt[:])
                        else:
                            nc.scalar.copy(ct[:, k, :], pt[:])
                    for k in range(KD):
                        nc.tensor.matmul(
                            sc[:], qt[:, k, :], ct[:, k, :],
                            start=(k == 0), stop=False,
                        )
                    # mask fold-in: K=1 row of ones^T @ madd (bf16)
                    base = b * S + c * CH
                    nc.tensor.matmul(
                        sc[:], ones[:], madd[:, base : base + CH],
                        start=False, stop=True,
                    )
                    # copy scores chunk to SBUF (ACT) + partial max (DVE)
                    nc.scalar.copy(ssb[:, c * CH : (c + 1) * CH], sc[:])
                    nc.vector.reduce_max(pmax[:, c : c + 1], sc[:], axis=AX.X)

                # softmax: exp(s - max) with fused sum; weights unnormalized
                negm = smp.tile([Q, 1], F32, tag="negm")
                nc.vector.tensor_reduce(
                    out=negm[:], in_=pmax[:], axis=AX.X, op=OP.max, negate=True
                )
                esb = scp.tile([Q, S], MM_DT, tag="sc_sb")
                ssum = smp.tile([Q, 1], F32, tag="ssum")
                nc.scalar.activation(
                    esb[:], ssb[:], ACTF.Exp,
                    bias=negm[:], scale=1.0, accum_out=ssum[:],
                )
                rinv = smp.tile([Q, 1], F32, tag="rinv")
                nc.vector.reciprocal(rinv[:], ssum[:])

                # second matmul: out[q, d] = sum_s w[s, q] * ctx[s, d]
                op0 = ps_o.tile([Q, 384], F32, tag="op0")
                op1 = ps_o.tile([Q, 384], F32, tag="op1")
                for c in range(NCH):
                    cn = cn_tiles[c]
                    wp = ps_t.tile([128, NSUB * Q], MM_DT, tag="pt")
                    for n in range(NSUB):
                        nc.tensor.transpose(
                            wp[:, n * Q : (n + 1) * Q],
                            esb[:, (c * NSUB + n) * 128 : (c * NSUB + n + 1) * 128],
                            ident[:Q, :Q],
                        )
                    wt = wtp.tile([128, NSUB, Q], MM_DT, tag="wt")
                    if c % 2 == 0:
                        nc.vector.tensor_copy(
                            wt[:].rearrange("p n q -> p (n q)"), wp[:]
                        )
                    else:
                        nc.scalar.copy(
                            wt[:].rearrange("p n q -> p (n q)"), wp[:]
                        )
                    for n in range(NSUB):
                        first = (c == 0 and n == 0)
                        last = (c == NCH - 1 and n == NSUB - 1)
                        nc.tensor.matmul(
                            op0[:], wt[:, n, :], cn[:, n, 0:384],
                            start=first, stop=last,
                        )
                        nc.tensor.matmul(
                            op1[:], wt[:, n, :], cn[:, n, 384:768],
                            start=first, stop=last,
                        )

                osb = smp.tile([Q, D], F32, tag="osb")
                nc.vector.tensor_scalar(
                    osb[:, 0:384], op0[:], rinv[:], None, op0=OP.mult
                )
                nc.vector.tensor_scalar(
                    osb[:, 384:768], op1[:], rinv[:], None, op0=OP.mult
                )
                nc.sync.dma_start(out_d.ap()[b], osb[:])

    nc.compile()
    return nc


_NC_CACHE = None


def _get_nc():
    global _NC_CACHE
    if _NC_CACHE is None:
        _NC_CACHE = build_nc()
    return _NC_CACHE


def kernel(context, attention_mask, query):
    context = np.ascontiguousarray(np.asarray(context, dtype=np.float32))
    attention_mask = np.ascontiguousarray(np.asarray(attention_mask, dtype=np.float32))
    query = np.ascontiguousarray(np.asarray(query, dtype=np.float32))
    assert context.shape == (B, S, D)
    assert attention_mask.shape == (B, S)
    assert query.shape == (Q, D)

    nc = _get_nc()
    in_maps = [
        {
            "context": context[i * BPC : (i + 1) * BPC],
            "attention_mask": attention_mask[i * BPC : (i + 1) * BPC],
            "query": query,
        }
        for i in range(NCORES)
    ]
    res = run_bass_kernel_spmd(nc, in_maps, list(range(NCORES)))
    out = np.concatenate(
        [res.results[i]["out"] for i in range(NCORES)], axis=0
    ).astype(np.float32)
    return out


# revision 21
# speedup vs baseline: 1.4198x; 1.0086x over previous
"""Attention-pooling (ContextLayer) Trainium2 Bass kernel, 8-core SPMD.

Computes, for full inputs:
    scores  = einsum('qd,bsd->bqs', query, context) + (1-mask)*NEG
    weights = softmax(scores, axis=-1)
    out     = einsum('bqs,bsd->bqd', weights, context)

Sharding: data-parallel over batch (16 batches -> 8 cores x 2), query
replicated. Per-core kernel is a single-pass (context read from HBM once)
streaming implementation:
  - context streamed in 8 chunks of 512 rows per batch, natural layout,
    retained in SBUF for the second matmul
  - PE transposes each chunk (D-on-partitions copy) for the scores matmul
  - scores matmul (fp32r) with query^T stationary; mask folded in as a
    K=1 bf16 accumulation row (ones^T @ (1-m)*NEG)
  - per-chunk scores copy to SBUF (ACT) + partial max (DVE)
  - exp(x-max) with fused sum on ACT, reciprocal on DVE
  - second matmul (fp32r) accumulates over all chunks in PSUM, normalized
    by 1/sum at the end.

fp32r notes: walrus requires every fp32r-matmul input to be *produced* as
float32r, so all matmul-feeding tiles are float32r and the PSUM->SBUF
copies perform the rounding; DMA-landed tiles (cn, qn) are declared
float32r and loaded via SWDGE with a nominal fp32->fp32r cast.
"""

import numpy as np

import concourse.bass as bass
import concourse.mybir as mybir
import concourse.tile as tile
from concourse import bacc
from concourse.bass_utils import run_bass_kernel_spmd
from concourse.masks import make_identity

# Problem shapes (hardcoded per contract)
B, S, D, Q = 16, 4096, 768, 64
NCORES = 8
BPC = B // NCORES          # batches per core
NEG = -100000.0

CH = 512                   # S-chunk rows per DMA
NCH = S // CH              # 8 chunks per batch
NSUB = CH // 128           # 4 128-row subtiles per chunk
KD = D // 128              # 6 contraction chunks over D
CN_BUFS = 10               # retained natural-context chunk slots (8 + prefetch)

F32 = mybir.dt.float32
BF16 = mybir.dt.bfloat16
MM_DT = mybir.dt.float32r  # matmul compute dtype (fp32 bits, fast PE path)


def build_nc():
    nc = bacc.Bacc("TRN2", target_bir_lowering=False, debug=False)

    ctx_d = nc.dram_tensor("context", [BPC, S, D], F32, kind="ExternalInput")
    msk_d = nc.dram_tensor("attention_mask", [BPC, S], F32, kind="ExternalInput")
    qry_d = nc.dram_tensor("query", [Q, D], F32, kind="ExternalInput")
    out_d = nc.dram_tensor("out", [BPC, Q, D], F32, kind="ExternalOutput")

    AX = mybir.AxisListType
    OP = mybir.AluOpType
    ACTF = mybir.ActivationFunctionType

    with tile.TileContext(nc) as tc:
        with (
            tc.tile_pool(name="const", bufs=1) as constp,
            tc.tile_pool(name="cn", bufs=CN_BUFS) as cnp,
            tc.tile_pool(name="ct", bufs=2) as ctp,
            tc.tile_pool(name="scores", bufs=2) as scp,
            tc.tile_pool(name="wt", bufs=2) as wtp,
            tc.tile_pool(name="small", bufs=2) as smp,
            tc.tile_pool(name="ps_t", bufs=4, space="PSUM") as ps_t,
            tc.tile_pool(name="ps_s", bufs=2, space="PSUM") as ps_s,
            tc.tile_pool(name="ps_o", bufs=1, space="PSUM") as ps_o,
        ):
            ident_f = constp.tile([128, 128], F32, tag="ident_f")
            make_identity(nc, ident_f[:])
            ident = constp.tile([128, 128], MM_DT, tag="ident")
            nc.vector.tensor_copy(ident[:], ident_f[:])

            # query^T: [128, 64] x6 built by PE transpose (fp32r)
            qn = constp.tile([Q, D], MM_DT, tag="qn")
            nc.sync.dma_start(qn[:], qry_d.ap().bitcast(MM_DT))
            qt = constp.tile([128, KD, Q], MM_DT, tag="qt")
            for k in range(KD):
                pq = ps_t.tile([128, Q], MM_DT, tag="pt")
                nc.tensor.transpose(
                    pq[:], qn[:, k * 128 : (k + 1) * 128], ident[:Q, :Q]
                )
                nc.vector.tensor_copy(qt[:, k, :], pq[:])

            # additive mask on one partition, folded into matmul1 as a K=1
            # bf16 accumulation row: scores += ones[1,Q]^T @ madd[1,S]
            madd = constp.tile([1, BPC * S], BF16, tag="madd")
            nc.gpsimd.dma_start(
                madd[:], msk_d.ap().rearrange("b s -> (b s)").unsqueeze(0)
            )  # f32 -> bf16 cast, simple 2D AP
            # in-place: (1-m)*NEG == m*(-NEG) + NEG  (exactly 0 for m==1; -1e5
            # rounds to -99840 in bf16 — same all-underflow contribution)
            nc.vector.tensor_scalar(
                madd[:], madd[:], -NEG, NEG, op0=OP.mult, op1=OP.add
            )
            ones = constp.tile([1, Q], BF16, tag="ones")
            nc.vector.memset(ones[:], 1.0)

            for b in range(BPC):
                ssb = scp.tile([Q, S], F32, tag="sc_sb")
                pmax = smp.tile([Q, NCH], F32, tag="pmax")
                cn_tiles = []
                for c in range(NCH):
                    cn = cnp.tile([128, NSUB, D], MM_DT, tag="cn")
                    cn_tiles.append(cn)
                    src = ctx_d.ap()[b, c * CH : (c + 1) * CH, :].rearrange(
                        "(n p) d -> p n d", p=128
                    )
                    nc.sync.dma_start(cn[:], src.bitcast(MM_DT))

                    sc = ps_s.tile([Q, CH], F32, tag="sc")
                    ct = ctp.tile([128, KD, CH], MM_DT, tag="ct")
                    for k in range(KD):
                        pt = ps_t.tile([128, CH], MM_DT, tag="pt")
                        for n in range(NSUB):
                            nc.tensor.transpose(
                                pt[:, n * 128 : (n + 1) * 128],
                                cn[:, n, k * 128 : (k + 1) * 128],
                                ident[:],
                            )
                        if k % 2 == 0:
                            nc.vector.tensor_copy(ct[:, k, :], pt[:])
                        else:
                            nc.scalar.copy(ct[:, k, :], pt[:])
                    for k in range(KD):
                        nc.tensor.matmul(
                            sc[:], qt[:, k, :], ct[:, k, :],
                            start=(k == 0), stop=False,
                        )
                    # mask fold-in: K=1 row of ones^T @ madd (bf16)
                    base = b * S + c * CH
                    nc.tensor.matmul(
                        sc[:], ones[:], madd[:, base : base + CH],
                        start=False, stop=True,
                    )
                    # copy scores chunk to SBUF (ACT) + partial max (DVE)
                    nc.scalar.copy(ssb[:, c * CH : (c + 1) * CH], sc[:])
                    nc.vector.reduce_max(pmax[:, c : c + 1], sc[:], axis=AX.X)

                # softmax: exp(s - max), chunked so mm2 starts ASAP; weights
                # stay unnormalized (normalize output at the end)
                negm = smp.tile([Q, 1], F32, tag="negm")
                nc.vector.tensor_reduce(
                    out=negm[:], in_=pmax[:], axis=AX.X, op=OP.max, negate=True
                )
                esb = scp.tile([Q, S], MM_DT, tag="sc_sb")
                psums = smp.tile([Q, NCH], F32, tag="psums")

                # second matmul: out[q, d] = sum_s w[s, q] * ctx[s, d]
                op0 = ps_o.tile([Q, 384], F32, tag="op0")
                op1 = ps_o.tile([Q, 384], F32, tag="op1")
                for c in range(NCH):
                    cn = cn_tiles[c]
                    nc.scalar.activation(
                        esb[:, c * CH : (c + 1) * CH],
                        ssb[:, c * CH : (c + 1) * CH],
                        ACTF.Exp,
                        bias=negm[:], scale=1.0,
                        accum_out=psums[:, c : c + 1],
                    )
                    wp = ps_t.tile([128, NSUB * Q], MM_DT, tag="pt")
                    for n in range(NSUB):
                        nc.tensor.transpose(
                            wp[:, n * Q : (n + 1) * Q],
                            esb[:, (c * NSUB + n) * 128 : (c * NSUB + n + 1) * 128],
                            ident[:Q, :Q],
                        )
                    wt = wtp.tile([128, NSUB, Q], MM_DT, tag="wt")
                    if c % 2 == 0:
                        nc.vector.tensor_copy(
                            wt[:].rearrange("p n q -> p (n q)"), wp[:]
                        )
                    else:
                        nc.scalar.copy(
                            wt[:].rearrange("p n q -> p (n q)"), wp[:]
                        )
                    for n in range(NSUB):
                        first = (c == 0 and n == 0)
                        last = (c == NCH - 1 and n == NSUB - 1)
                        nc.tensor.matmul(
                            op0[:], wt[:, n, :], cn[:, n, 0:384],
                            start=first, stop=last,
                        )
                        nc.tensor.matmul(
                            op1[:], wt[:, n, :], cn[:, n, 384:768],
                            start=first, stop=last,
                        )

                ssum = smp.tile([Q, 1], F32, tag="ssum")
                nc.vector.reduce_sum(ssum[:], psums[:], axis=AX.X)
                rinv = smp.tile([Q, 1], F32, tag="rinv")
                nc.vector.reciprocal(rinv[:], ssum[:])

                osb = smp.tile([Q, D], F32, tag="osb")
                nc.vector.tensor_scalar(
                    osb[:, 0:384], op0[:], rinv[:], None, op0=OP.mult
                )
                nc.vector.tensor_scalar(
                    osb[:, 384:768], op1[:], rinv[:], None, op0=OP.mult
                )
                nc.sync.dma_start(out_d.ap()[b], osb[:])

    nc.compile()
    return nc


_NC_CACHE = None


def _get_nc():
    global _NC_CACHE
    if _NC_CACHE is None:
        _NC_CACHE = build_nc()
    return _NC_CACHE


def kernel(context, attention_mask, query):
    context = np.ascontiguousarray(np.asarray(context, dtype=np.float32))
    attention_mask = np.ascontiguousarray(np.asarray(attention_mask, dtype=np.float32))
    query = np.ascontiguousarray(np.asarray(query, dtype=np.float32))
    assert context.shape == (B, S, D)
    assert attention_mask.shape == (B, S)
    assert query.shape == (Q, D)

    nc = _get_nc()
    in_maps = [
        {
            "context": context[i * BPC : (i + 1) * BPC],
            "attention_mask": attention_mask[i * BPC : (i + 1) * BPC],
            "query": query,
        }
        for i in range(NCORES)
    ]
    res = run_bass_kernel_spmd(nc, in_maps, list(range(NCORES)))
    out = np.concatenate(
        [res.results[i]["out"] for i in range(NCORES)], axis=0
    ).astype(np.float32)
    return out


# revision 22
# speedup vs baseline: 1.4309x; 1.0078x over previous
"""Attention-pooling (ContextLayer) Trainium2 Bass kernel, 8-core SPMD.

Computes, for full inputs:
    scores  = einsum('qd,bsd->bqs', query, context) + (1-mask)*NEG
    weights = softmax(scores, axis=-1)
    out     = einsum('bqs,bsd->bqd', weights, context)

Sharding: data-parallel over batch (16 batches -> 8 cores x 2), query
replicated. Per-core kernel is a single-pass (context read from HBM once)
streaming implementation:
  - context streamed in 8 chunks of 512 rows per batch, natural layout,
    retained in SBUF for the second matmul
  - PE transposes each chunk (D-on-partitions copy) for the scores matmul
  - scores matmul (fp32r) with query^T stationary; mask folded in as a
    K=1 bf16 accumulation row (ones^T @ (1-m)*NEG)
  - per-chunk scores copy to SBUF (ACT) + partial max (DVE)
  - exp(x-max) with fused sum on ACT, reciprocal on DVE
  - second matmul (fp32r) accumulates over all chunks in PSUM, normalized
    by 1/sum at the end.

fp32r notes: walrus requires every fp32r-matmul input to be *produced* as
float32r, so all matmul-feeding tiles are float32r and the PSUM->SBUF
copies perform the rounding; DMA-landed tiles (cn, qn) are declared
float32r and loaded via SWDGE with a nominal fp32->fp32r cast.
"""

import numpy as np

import concourse.bass as bass
import concourse.mybir as mybir
import concourse.tile as tile
from concourse import bacc
from concourse.bass_utils import run_bass_kernel_spmd
from concourse.masks import make_identity

# Problem shapes (hardcoded per contract)
B, S, D, Q = 16, 4096, 768, 64
NCORES = 8
BPC = B // NCORES          # batches per core
NEG = -100000.0

CH = 512                   # S-chunk rows per DMA
NCH = S // CH              # 8 chunks per batch
NSUB = CH // 128           # 4 128-row subtiles per chunk
KD = D // 128              # 6 contraction chunks over D
CN_BUFS = 10               # retained natural-context chunk slots (8 + prefetch)

F32 = mybir.dt.float32
BF16 = mybir.dt.bfloat16
MM_DT = mybir.dt.float32r  # matmul compute dtype (fp32 bits, fast PE path)


def build_nc():
    nc = bacc.Bacc("TRN2", target_bir_lowering=False, debug=False)

    ctx_d = nc.dram_tensor("context", [BPC, S, D], F32, kind="ExternalInput")
    msk_d = nc.dram_tensor("attention_mask", [BPC, S], F32, kind="ExternalInput")
    qry_d = nc.dram_tensor("query", [Q, D], F32, kind="ExternalInput")
    out_d = nc.dram_tensor("out", [BPC, Q, D], F32, kind="ExternalOutput")

    AX = mybir.AxisListType
    OP = mybir.AluOpType
    ACTF = mybir.ActivationFunctionType

    with tile.TileContext(nc) as tc:
        with (
            tc.tile_pool(name="const", bufs=1) as constp,
            tc.tile_pool(name="cn", bufs=CN_BUFS) as cnp,
            tc.tile_pool(name="ct", bufs=2) as ctp,
            tc.tile_pool(name="scores", bufs=2) as scp,
            tc.tile_pool(name="wt", bufs=2) as wtp,
            tc.tile_pool(name="small", bufs=2) as smp,
            tc.tile_pool(name="ps_t", bufs=4, space="PSUM") as ps_t,
            tc.tile_pool(name="ps_s", bufs=2, space="PSUM") as ps_s,
            tc.tile_pool(name="ps_o", bufs=1, space="PSUM") as ps_o,
        ):
            ident_f = constp.tile([128, 128], F32, tag="ident_f")
            make_identity(nc, ident_f[:])
            ident = constp.tile([128, 128], MM_DT, tag="ident")
            nc.vector.tensor_copy(ident[:], ident_f[:])

            # query^T: [128, 64] x6 built by PE transpose (fp32r)
            qn = constp.tile([Q, D], MM_DT, tag="qn")
            nc.sync.dma_start(qn[:], qry_d.ap().bitcast(MM_DT))
            qt = constp.tile([128, KD, Q], MM_DT, tag="qt")
            for k in range(KD):
                pq = ps_t.tile([128, Q], MM_DT, tag="pt")
                nc.tensor.transpose(
                    pq[:], qn[:, k * 128 : (k + 1) * 128], ident[:Q, :Q]
                )
                nc.vector.tensor_copy(qt[:, k, :], pq[:])

            # additive mask on one partition, folded into matmul1 as a K=1
            # bf16 accumulation row: scores += ones[1,Q]^T @ madd[1,S]
            madd = constp.tile([1, BPC * S], BF16, tag="madd")
            nc.gpsimd.dma_start(
                madd[:], msk_d.ap().rearrange("b s -> (b s)").unsqueeze(0)
            )  # f32 -> bf16 cast, simple 2D AP
            # in-place: (1-m)*NEG == m*(-NEG) + NEG  (exactly 0 for m==1; -1e5
            # rounds to -99840 in bf16 — same all-underflow contribution)
            nc.vector.tensor_scalar(
                madd[:], madd[:], -NEG, NEG, op0=OP.mult, op1=OP.add
            )
            ones = constp.tile([1, Q], BF16, tag="ones")
            nc.vector.memset(ones[:], 1.0)

            for b in range(BPC):
                ssb = scp.tile([Q, S], F32, tag="sc_sb")
                pmax = smp.tile([Q, NCH], F32, tag="pmax")
                cn_tiles = []
                for c in range(NCH):
                    cn = cnp.tile([128, NSUB, D], MM_DT, tag="cn")
                    cn_tiles.append(cn)
                    if b == 0 and c < 2:
                        # ramp: per-subtile DMAs so the first transposes can
                        # start as soon as 128 rows have landed
                        for n in range(NSUB):
                            r0 = c * CH + n * 128
                            nc.sync.dma_start(
                                cn[:, n, :],
                                ctx_d.ap()[b, r0 : r0 + 128, :].bitcast(MM_DT),
                            )
                    else:
                        src = ctx_d.ap()[b, c * CH : (c + 1) * CH, :].rearrange(
                            "(n p) d -> p n d", p=128
                        )
                        nc.sync.dma_start(cn[:], src.bitcast(MM_DT))

                    # interleave transpose groups with the scores matmuls so
                    # the PE stream always carries real matmuls (HAM warmth)
                    sc = ps_s.tile([Q, CH], F32, tag="sc")
                    ct = ctp.tile([128, KD, CH], MM_DT, tag="ct")
                    for k in range(KD):
                        pt = ps_t.tile([128, CH], MM_DT, tag="pt")
                        for n in range(NSUB):
                            nc.tensor.transpose(
                                pt[:, n * 128 : (n + 1) * 128],
                                cn[:, n, k * 128 : (k + 1) * 128],
                                ident[:],
                            )
                        if k % 2 == 0:
                            nc.vector.tensor_copy(ct[:, k, :], pt[:])
                        else:
                            nc.scalar.copy(ct[:, k, :], pt[:])
                        if k >= 1:
                            nc.tensor.matmul(
                                sc[:], qt[:, k - 1, :], ct[:, k - 1, :],
                                start=(k == 1), stop=False,
                            )
                    nc.tensor.matmul(
                        sc[:], qt[:, KD - 1, :], ct[:, KD - 1, :],
                        start=False, stop=False,
                    )
                    # mask fold-in: K=1 row of ones^T @ madd (bf16)
                    base = b * S + c * CH
                    nc.tensor.matmul(
                        sc[:], ones[:], madd[:, base : base + CH],
                        start=False, stop=True,
                    )
                    # copy scores chunk to SBUF (ACT) + partial max (DVE)
                    nc.scalar.copy(ssb[:, c * CH : (c + 1) * CH], sc[:])
                    nc.vector.reduce_max(pmax[:, c : c + 1], sc[:], axis=AX.X)

                # softmax: exp(s - max), chunked so mm2 starts ASAP; weights
                # stay unnormalized (normalize output at the end)
                negm = smp.tile([Q, 1], F32, tag="negm")
                nc.vector.tensor_reduce(
                    out=negm[:], in_=pmax[:], axis=AX.X, op=OP.max, negate=True
                )
                esb = scp.tile([Q, S], MM_DT, tag="sc_sb")
                psums = smp.tile([Q, NCH], F32, tag="psums")

                # second matmul: out[q, d] = sum_s w[s, q] * ctx[s, d]
                op0 = ps_o.tile([Q, 384], F32, tag="op0")
                op1 = ps_o.tile([Q, 384], F32, tag="op1")
                for c in range(NCH):
                    cn = cn_tiles[c]
                    nc.scalar.activation(
                        esb[:, c * CH : (c + 1) * CH],
                        ssb[:, c * CH : (c + 1) * CH],
                        ACTF.Exp,
                        bias=negm[:], scale=1.0,
                        accum_out=psums[:, c : c + 1],
                    )
                    wp = ps_t.tile([128, NSUB * Q], MM_DT, tag="pt")
                    for n in range(NSUB):
                        nc.tensor.transpose(
                            wp[:, n * Q : (n + 1) * Q],
                            esb[:, (c * NSUB + n) * 128 : (c * NSUB + n + 1) * 128],
                            ident[:Q, :Q],
                        )
                    wt = wtp.tile([128, NSUB, Q], MM_DT, tag="wt")
                    if c % 2 == 0:
                        nc.vector.tensor_copy(
                            wt[:].rearrange("p n q -> p (n q)"), wp[:]
                        )
                    else:
                        nc.scalar.copy(
                            wt[:].rearrange("p n q -> p (n q)"), wp[:]
                        )
                    for n in range(NSUB):
                        first = (c == 0 and n == 0)
                        last = (c == NCH - 1 and n == NSUB - 1)
                        nc.tensor.matmul(
                            op0[:], wt[:, n, :], cn[:, n, 0:384],
                            start=first, stop=last,
                        )
                        nc.tensor.matmul(
                            op1[:], wt[:, n, :], cn[:, n, 384:768],
                            start=first, stop=last,
                        )

                ssum = smp.tile([Q, 1], F32, tag="ssum")
                nc.vector.reduce_sum(ssum[:], psums[:], axis=AX.X)
                rinv = smp.tile([Q, 1], F32, tag="rinv")
                nc.vector.reciprocal(rinv[:], ssum[:])

                osb = smp.tile([Q, D], F32, tag="osb")
                nc.vector.tensor_scalar(
                    osb[:, 0:384], op0[:], rinv[:], None, op0=OP.mult
                )
                nc.vector.tensor_scalar(
                    osb[:, 384:768], op1[:], rinv[:], None, op0=OP.mult
                )
                nc.sync.dma_start(out_d.ap()[b], osb[:])

    nc.compile()
    return nc


_NC_CACHE = None


def _get_nc():
    global _NC_CACHE
    if _NC_CACHE is None:
        _NC_CACHE = build_nc()
    return _NC_CACHE


def kernel(context, attention_mask, query):
    context = np.ascontiguousarray(np.asarray(context, dtype=np.float32))
    attention_mask = np.ascontiguousarray(np.asarray(attention_mask, dtype=np.float32))
    query = np.ascontiguousarray(np.asarray(query, dtype=np.float32))
    assert context.shape == (B, S, D)
    assert attention_mask.shape == (B, S)
    assert query.shape == (Q, D)

    nc = _get_nc()
    in_maps = [
        {
            "context": context[i * BPC : (i + 1) * BPC],
            "attention_mask": attention_mask[i * BPC : (i + 1) * BPC],
            "query": query,
        }
        for i in range(NCORES)
    ]
    res = run_bass_kernel_spmd(nc, in_maps, list(range(NCORES)))
    out = np.concatenate(
        [res.results[i]["out"] for i in range(NCORES)], axis=0
    ).astype(np.float32)
    return out
